# revision 55
# baseline (speedup 1.0000x reference)
# Trainium2 Bass kernel: 2:4 structured activation pruning + Linear.
#
#   out = magnitude_prune_2of4(x.reshape(-1, 4096)) @ weight.T
#
# Sharding: data-parallel over the flattened token dim (16384 tokens ->
# 2048/core across 8 cores); weight replicated (host-permuted + bf16 so
# the contraction dim lands on SBUF partitions). No collectives.
#
# Pipeline (PE does ONLY matmuls; transposes run on the DMA xbar):
#   host: x/weight -> bf16 (+ bf16-level tie fix, exact decisions)
#   -> DMA x (bf16) -> DVE |x| pairwise max/min (custom ops) -> DVE
#   compact tree -> per-group-of-4 2nd-max threshold (exact fp32 over the
#   bf16 values) -> DVE prune-select -> DMA-xbar SBUF->SBUF transpose
#   -> PE bf16 matmuls (FWL weight loads) accumulating over 32 d-chunks
#   -> ACT PSUM->SBUF copy w/ bf16 cast -> DMA out (bf16, host-upcast).
#
# Scheduling notes (hardware-trace driven; see the memory file
# trn2-tile-scheduler-dma-serialization for the full story): the tile
# scheduler serializes ALL DMAs through one modeled resource, so the 8MB
# weight ships as 8 x 1MB pieces created after tile 0's spans (exactly 8:
# one per DMAHW lane, so no piece lane-waits on a transpose); PSUM
# accumulation groups stay in contiguous 16-matmul bursts; loads +
# transposes share the sync HWDGE queue, PSUM copies the scalar queue,
# stores the gpsimd SWDGE queue.
import numpy as np

N_CORES = 8
BS, SEQ, D = 4, 4096, 4096
OUTF = 1024
TOK_TOTAL = BS * SEQ
TOK = TOK_TOTAL // N_CORES      # 2048 tokens per core
P = 128                         # SBUF partitions
NT = TOK // P                   # 16 token tiles per core
HALF = D // 2                   # 2048: free-dim half width
NCH = D // P                    # 32 d-chunks of 128
NCH_H = NCH // 2                # 16 d-chunks per half

_compiled = None
_custom_ops = None


def _register_custom_dve():
    # Fused DVE ops (registered into the runtime op table; compiled into the
    # per-NEFF DVE table): pairwise abs-max/abs-min, and the pruning select
    # out = |x| >= thr ? x : 0. Halves DVE work vs stock-op sequences.
    global _custom_ops
    if _custom_ops is not None:
        return _custom_ops
    from concourse import dve_ops as D
    from concourse.dve_spec import Spec, Src0, Src1, Zero, maxx, minn, select, lower
    from concourse.dve_uop import DveOpSpec

    def mk(name, body, reference):
        spec = Spec(body=body, reference=reference)
        shas = {}
        for ver in ("v3", "v4"):
            try:
                u = lower(spec, ver=ver)
                shas[ver] = DveOpSpec(name=name, opcode=1, uops=u,
                                      rd1_en=True).sha(ver)
            except Exception:
                if ver == "v3":
                    raise
        return D.DveOp(name=name, spec=spec, subdim=False, uops_sha=shas)

    absa = maxx(Src0, Zero - Src0)
    absb = maxx(Src1, Zero - Src1)
    ops = (
        mk("ABS_MAX2_ANT", maxx(absa, absb),
           lambda in0, in1, *a: np.maximum(np.abs(in0), np.abs(in1))),
        mk("ABS_MIN2_ANT", minn(absa, absb),
           lambda in0, in1, *a: np.minimum(np.abs(in0), np.abs(in1))),
        mk("PRUNE24_ANT", select(maxx(Src0, Zero - Src0) >= Src1, Src0, Zero),
           lambda in0, in1, *a: np.where(np.abs(in0) >= in1, in0, 0.0)),
    )
    for op in ops:
        if op.name not in D._SUB_OPCODE_FOR_NAME:
            D.OPS.append(op)
            D.CUSTOM_DVE_SPECS[op.name] = op.spec
            D._SUB_OPCODE_FOR_NAME[op.name] = (
                D._CUSTOM_DVE_ROW_BASE + len(D._SUB_OPCODE_FOR_NAME))
    _custom_ops = ops
    return ops


def _build():
    import concourse.tile as tile
    import concourse.mybir as mybir
    from concourse import bacc

    ABS_MAX2, ABS_MIN2, PRUNE24 = _register_custom_dve()
    f32 = mybir.dt.float32
    bf16 = mybir.dt.bfloat16
    Alu = mybir.AluOpType

    nc = bacc.Bacc("TRN2", target_bir_lowering=False, debug=False,
                   num_devices=N_CORES)
    xs_ap = nc.dram_tensor("xs", [TOK, D], bf16, kind="ExternalInput").ap()
    # weight.T host-permuted to [p, c, o] (d = 128c + p) so each partition's
    # DMA source is one contiguous 64KB run -- few big descriptors.
    wt_ap = nc.dram_tensor("wt", [P, NCH, OUTF], bf16,
                           kind="ExternalInput").ap()
    o_ap = nc.dram_tensor("o", [TOK, OUTF], bf16, kind="ExternalOutput").ap()

    with tile.TileContext(nc) as tc:
        with tc.tile_pool(name="wpool", bufs=1) as wpool, \
             tc.tile_pool(name="xin", bufs=3) as xin, \
             tc.tile_pool(name="mwork", bufs=2) as mwork, \
             tc.tile_pool(name="twork", bufs=2) as twork, \
             tc.tile_pool(name="spool", bufs=2) as spool, \
             tc.tile_pool(name="xtp", bufs=2) as xtp, \
             tc.tile_pool(name="outp", bufs=2) as outp, \
             tc.tile_pool(name="pso", bufs=4, space="PSUM") as pso:

            def process_span(xh_view, xt_dst, w):
                # prune a [P, w] bf16 span (thresholds exact in fp32 on the
                # bf16 values) and deposit the transposed bf16 chunks into
                # xt_dst ([P, w//128, P]) via the DMA xbar.
                # pairwise tree: thr = 2nd-largest |x| per group of 4
                x2 = xh_view.rearrange("p (g two) -> p g two", two=2)
                mx = mwork.tile([P, w // 2], f32, tag="mx",
                                padded_shape=[P, HALF // 2])
                mn = mwork.tile([P, w // 2], f32, tag="mn",
                                padded_shape=[P, HALF // 2])
                nc.vector._custom_dve(ABS_MAX2, out=mx,
                                      in0=x2[:, :, 0], in1=x2[:, :, 1])
                nc.vector._custom_dve(ABS_MIN2, out=mn,
                                      in0=x2[:, :, 0], in1=x2[:, :, 1])
                # compact: 2nd-max = max(min of pair-maxes, max of pair-mins)
                mx2 = mx.rearrange("p (g two) -> p g two", two=2)
                mn2 = mn.rearrange("p (g two) -> p g two", two=2)
                mm = twork.tile([P, w // 4], f32, tag="mm",
                                padded_shape=[P, HALF // 4])
                nm = twork.tile([P, w // 4], f32, tag="nm",
                                padded_shape=[P, HALF // 4])
                nc.vector.tensor_tensor(mm, mx2[:, :, 0], mx2[:, :, 1], Alu.min)
                nc.vector.tensor_tensor(nm, mn2[:, :, 0], mn2[:, :, 1], Alu.max)
                thr = mm
                nc.vector.tensor_tensor(thr, mm, nm, Alu.max)
                # prune: xspr = |x| >= thr ? x : 0, cast to bf16 on write
                thr_b = thr.unsqueeze(2).broadcast_to([P, w // 4, 4])
                xspr = spool.tile([P, w], bf16, tag="xspr",
                                  padded_shape=[P, HALF])
                nc.vector._custom_dve(
                    PRUNE24,
                    out=xspr.rearrange("p (g four) -> p g four", four=4),
                    in0=xh_view.rearrange("p (g four) -> p g four", four=4),
                    in1=thr_b)
                # SBUF->SBUF 128x128 transposes via the DMA xbar:
                # out[p, c, t] = xspr[t, 128c + p].  On the sync queue with
                # the x loads (same producer-side dependency cadence) so
                # they never queue behind PSUM-copy semaphore waits.
                nc.sync.dma_start(out=xt_dst, in_=xspr, transpose=True)

            w_pieces = []
            QRT = D // 4
            NCH_Q = QRT // P        # 8 d-chunks per quarter
            for i in range(NT):
                if i == 0:
                    # Tile 0 runs at QUARTER granularity: the first
                    # transpose is ready ~7us into the schedule, so it (and
                    # the first matmul burst) slots between the weight
                    # pieces instead of behind them, cutting the ramp.
                    xts = []
                    for q in range(4):
                        xq = xin.tile([P, QRT], bf16, tag="xh",
                                      padded_shape=[P, D])
                        nc.sync.dma_start(
                            out=xq, in_=xs_ap[0:P, q * QRT:(q + 1) * QRT])
                        xtq = xtp.tile([P, NCH_Q, P], bf16,
                                       tag=f"xt{q % 2}",
                                       padded_shape=[P, NCH_H, P])
                        process_span(xq, xtq, QRT)
                        xts.append(xtq)
                    spans = [(xts[q], NCH_Q, q == 0, q == 3)
                             for q in range(4)]
                else:
                    # one bf16 x load per tile; transposed pruned
                    # activations deposited per half (separate tiles so
                    # half-0 matmuls don't wait on the half-1 transpose).
                    xh = xin.tile([P, D], bf16, tag="xh")
                    nc.sync.dma_start(out=xh,
                                      in_=xs_ap[i * P:(i + 1) * P, :])
                    xt0 = xtp.tile([P, NCH_H, P], bf16, tag="xt0")
                    xt1 = xtp.tile([P, NCH_H, P], bf16, tag="xt1")
                    for h in range(2):
                        process_span(xh[:, h * HALF:(h + 1) * HALF],
                                     [xt0, xt1][h], HALF)
                    spans = [(xt0, NCH_H, True, False),
                             (xt1, NCH_H, False, True)]

                if i == 0:
                    # weight resident in SBUF as 8 x 1MB piece tiles
                    # [P, 4, OUTF] bf16 (piece j = d-chunks 4j..4j+3) on the
                    # scalar HWDGE queue, created AFTER tile 0's spans.  The
                    # scheduler's cost model serializes all DMAs through one
                    # exclusive resource; small pieces let the ready (and
                    # higher-priority) xbar transposes slot between them
                    # instead of stalling behind one 8MB transfer.  Exactly
                    # 8 pieces: each lands on a distinct DMAHW lane whose
                    # predecessor is an early fast DMA -- with 16 pieces the
                    # later ones lane-wait on the first transposes, which
                    # chain-wait on the pieces, delaying both the weight
                    # completion and the PE start.
                    for j in range(NCH // 4):
                        w_p = wpool.tile([P, 4, OUTF], bf16, tag=f"w{j}")
                        nc.scalar.dma_start(
                            out=w_p, in_=wt_ap[:, 4 * j:4 * j + 4, :])
                        w_pieces.append(w_p)

                # matmul: psum[tok, outf-half] += xt[cc].T @ wT[cc].
                # span-outer at >=8-matmul burst granularity: both banks'
                # early-span bursts run before any later-span burst, so the
                # PE can proceed while later transposes are in flight.  Each
                # bank's accumulation group stays in contiguous bursts
                # (per-matmul bank alternation triggers the PSUM-cycling HAM
                # degradation).
                osb = outp.tile([P, OUTF], bf16, tag="osb")
                pout0 = pso.tile([P, OUTF // 2], f32, tag="ps0", bufs=2)
                pout1 = pso.tile([P, OUTF // 2], f32, tag="ps1", bufs=2)
                pouts = [pout0, pout1]
                cc = 0
                for xt_s, nch_s, first, last in spans:
                    for n in range(2):
                        for c in range(nch_s):
                            nc.tensor.matmul(
                                pouts[n],
                                xt_s[:, c, :],
                                w_pieces[(cc + c) // 4][:, (cc + c) % 4,
                                                        n * 512:(n + 1) * 512],
                                start=(first and c == 0),
                                stop=(last and c == nch_s - 1))
                    cc += nch_s
                for n in range(2):
                    # PSUM f32 -> SBUF bf16 cast on the ACT copy
                    nc.scalar.copy(osb[:, n * 512:(n + 1) * 512], pouts[n])
                # one bf16 store per tile on the SWDGE queue
                nc.gpsimd.dma_start(out=o_ap[i * P:(i + 1) * P, :], in_=osb)
    nc.compile()
    return nc


def _get_compiled():
    global _compiled
    if _compiled is None:
        _compiled = _build()
    return _compiled


def _fix_ties(x_flat):
    # The device prunes on bf16 values: it keeps elements with
    # |bf16(x)| >= (2nd-largest |bf16(x)| of the group).  Where that
    # decision differs from the reference (fp32 top_k, stable tie-break) --
    # i.e. groups whose 2nd and 3rd magnitudes collapse to the same bf16 --
    # pre-zero the reference-DROPPED elements so the device's threshold
    # test keeps exactly the reference-kept pair.  The zeroed elements are
    # dropped by the reference either way, so values are unaffected.
    import ml_dtypes
    ab = np.abs(x_flat.astype(ml_dtypes.bfloat16).astype(np.float32))
    g = ab.reshape(-1, 4)
    m1 = np.maximum(g[:, 0], g[:, 1]); n1 = np.minimum(g[:, 0], g[:, 1])
    m2 = np.maximum(g[:, 2], g[:, 3]); n2 = np.minimum(g[:, 2], g[:, 3])
    thr = np.maximum(np.minimum(m1, m2), np.maximum(n1, n2))
    third = np.minimum(np.minimum(m1, m2), np.maximum(n1, n2))
    tied = np.flatnonzero(thr == third)
    if len(tied) == 0:
        return x_flat
    x_flat = x_flat.copy()
    gv = x_flat.reshape(-1, 4)
    rows = gv[tied]
    # reference keep-set: top-2 of fp32 |x|, stable order
    order = np.argsort(-np.abs(rows), axis=1, kind="stable")
    np.put_along_axis(rows, order[:, 2:], 0.0, axis=1)
    gv[tied] = rows
    return x_flat


def _prep_inputs(x: np.ndarray, weight: np.ndarray) -> list:
    import ml_dtypes
    x_flat = np.ascontiguousarray(x.reshape(TOK_TOTAL, D), dtype=np.float32)
    x_flat = _fix_ties(x_flat)
    xs16 = np.ascontiguousarray(x_flat.astype(ml_dtypes.bfloat16))
    # weight.T permuted to [p, c, o] with d = 128c + p, so the device DMA
    # reads one contiguous 64KB run per partition.
    wt = np.ascontiguousarray(
        weight.T.astype(ml_dtypes.bfloat16)
        .reshape(NCH, P, OUTF).transpose(1, 0, 2))
    return [{"xs": xs16[c * TOK:(c + 1) * TOK], "wt": wt}
            for c in range(N_CORES)]


def kernel(x: np.ndarray, weight: np.ndarray) -> np.ndarray:
    from concourse.bass_utils import run_bass_kernel_spmd

    nc = _get_compiled()
    in_maps = _prep_inputs(x, weight)
    res = run_bass_kernel_spmd(nc, in_maps, core_ids=list(range(N_CORES)))
    out = np.concatenate([res.results[c]["o"] for c in range(N_CORES)],
                         axis=0).astype(np.float32)
    return out.reshape(BS, SEQ, OUTF)


# revision 57
# speedup vs baseline: 1.0513x; 1.0513x over previous
# Trainium2 Bass kernel: 2:4 structured activation pruning + Linear.
#
#   out = magnitude_prune_2of4(x.reshape(-1, 4096)) @ weight.T
#
# Sharding: data-parallel over the flattened token dim (16384 tokens ->
# 2048/core across 8 cores); weight replicated (host-permuted + bf16 so
# the contraction dim lands on SBUF partitions). No collectives.
#
# Pipeline (PE does ONLY matmuls; transposes run on the DMA xbar):
#   host: x/weight -> bf16 (+ bf16-level tie fix, exact decisions)
#   -> DMA x (bf16) -> DVE |x| pairwise max/min (custom ops) -> DVE
#   compact tree -> per-group-of-4 2nd-max threshold (exact fp32 over the
#   bf16 values) -> DVE prune-select -> DMA-xbar SBUF->SBUF transpose
#   -> PE bf16 matmuls (FWL weight loads) accumulating over 32 d-chunks
#   -> ACT PSUM->SBUF copy w/ bf16 cast -> DMA out (bf16, host-upcast).
#
# Scheduling notes (hardware-trace driven; see the memory file
# trn2-tile-scheduler-dma-serialization for the full story): the tile
# scheduler serializes ALL DMAs through one modeled resource, so the 8MB
# weight ships as 8 x 1MB pieces created after tile 0's spans (exactly 8:
# one per DMAHW lane, so no piece lane-waits on a transpose); PSUM
# accumulation groups stay in contiguous 16-matmul bursts; loads +
# transposes share the sync HWDGE queue, PSUM copies the scalar queue,
# stores the gpsimd SWDGE queue.
import numpy as np

N_CORES = 8
BS, SEQ, D = 4, 4096, 4096
OUTF = 1024
TOK_TOTAL = BS * SEQ
TOK = TOK_TOTAL // N_CORES      # 2048 tokens per core
P = 128                         # SBUF partitions
NT = TOK // P                   # 16 token tiles per core
HALF = D // 2                   # 2048: free-dim half width
NCH = D // P                    # 32 d-chunks of 128
NCH_H = NCH // 2                # 16 d-chunks per half

_compiled = None
_custom_ops = None


def _register_custom_dve():
    # Fused DVE ops (registered into the runtime op table; compiled into the
    # per-NEFF DVE table): pairwise abs-max/abs-min, and the pruning select
    # out = |x| >= thr ? x : 0. Halves DVE work vs stock-op sequences.
    global _custom_ops
    if _custom_ops is not None:
        return _custom_ops
    from concourse import dve_ops as D
    from concourse.dve_spec import Spec, Src0, Src1, Zero, maxx, minn, select, lower
    from concourse.dve_uop import DveOpSpec

    def mk(name, body, reference):
        spec = Spec(body=body, reference=reference)
        shas = {}
        for ver in ("v3", "v4"):
            try:
                u = lower(spec, ver=ver)
                shas[ver] = DveOpSpec(name=name, opcode=1, uops=u,
                                      rd1_en=True).sha(ver)
            except Exception:
                if ver == "v3":
                    raise
        return D.DveOp(name=name, spec=spec, subdim=False, uops_sha=shas)

    absa = maxx(Src0, Zero - Src0)
    absb = maxx(Src1, Zero - Src1)
    ops = (
        mk("ABS_MAX2_ANT", maxx(absa, absb),
           lambda in0, in1, *a: np.maximum(np.abs(in0), np.abs(in1))),
        mk("ABS_MIN2_ANT", minn(absa, absb),
           lambda in0, in1, *a: np.minimum(np.abs(in0), np.abs(in1))),
        mk("PRUNE24_ANT", select(maxx(Src0, Zero - Src0) >= Src1, Src0, Zero),
           lambda in0, in1, *a: np.where(np.abs(in0) >= in1, in0, 0.0)),
    )
    for op in ops:
        if op.name not in D._SUB_OPCODE_FOR_NAME:
            D.OPS.append(op)
            D.CUSTOM_DVE_SPECS[op.name] = op.spec
            D._SUB_OPCODE_FOR_NAME[op.name] = (
                D._CUSTOM_DVE_ROW_BASE + len(D._SUB_OPCODE_FOR_NAME))
    _custom_ops = ops
    return ops


def _build():
    import concourse.tile as tile
    import concourse.mybir as mybir
    from concourse import bacc

    ABS_MAX2, ABS_MIN2, PRUNE24 = _register_custom_dve()
    f32 = mybir.dt.float32
    bf16 = mybir.dt.bfloat16
    Alu = mybir.AluOpType

    nc = bacc.Bacc("TRN2", target_bir_lowering=False, debug=False,
                   num_devices=N_CORES)
    xs_ap = nc.dram_tensor("xs", [TOK, D], bf16, kind="ExternalInput").ap()
    # weight.T host-permuted to [p, c, o] (d = 128c + p) so each partition's
    # DMA source is one contiguous 64KB run -- few big descriptors.
    wt_ap = nc.dram_tensor("wt", [P, NCH, OUTF], bf16,
                           kind="ExternalInput").ap()
    o_ap = nc.dram_tensor("o", [TOK, OUTF], bf16, kind="ExternalOutput").ap()

    with tile.TileContext(nc) as tc:
        with tc.tile_pool(name="wpool", bufs=1) as wpool, \
             tc.tile_pool(name="xin", bufs=4) as xin, \
             tc.tile_pool(name="mwork", bufs=2) as mwork, \
             tc.tile_pool(name="twork", bufs=2) as twork, \
             tc.tile_pool(name="spool", bufs=2) as spool, \
             tc.tile_pool(name="xtp", bufs=2) as xtp, \
             tc.tile_pool(name="outp", bufs=2) as outp, \
             tc.tile_pool(name="pso", bufs=4, space="PSUM") as pso:

            def process_span(i, xt, xh, h):
                # prune x[i-tile, h-half] (bf16; thresholds exact in fp32 on
                # the bf16 values) and deposit the transposed bf16 chunks
                # into xt[:, h*16:(h+1)*16, :] via the DMA xbar.
                xh = xh[:, h * HALF:(h + 1) * HALF]
                # pairwise tree: thr = 2nd-largest |x| per group of 4
                x2 = xh.rearrange("p (g two) -> p g two", two=2)
                mx = mwork.tile([P, HALF // 2], f32, tag="mx")
                mn = mwork.tile([P, HALF // 2], f32, tag="mn")
                nc.vector._custom_dve(ABS_MAX2, out=mx,
                                      in0=x2[:, :, 0], in1=x2[:, :, 1])
                nc.vector._custom_dve(ABS_MIN2, out=mn,
                                      in0=x2[:, :, 0], in1=x2[:, :, 1])
                # compact: 2nd-max = max(min of pair-maxes, max of pair-mins)
                mx2 = mx.rearrange("p (g two) -> p g two", two=2)
                mn2 = mn.rearrange("p (g two) -> p g two", two=2)
                mm = twork.tile([P, HALF // 4], f32, tag="mm")
                nm = twork.tile([P, HALF // 4], f32, tag="nm")
                nc.vector.tensor_tensor(mm, mx2[:, :, 0], mx2[:, :, 1], Alu.min)
                nc.vector.tensor_tensor(nm, mn2[:, :, 0], mn2[:, :, 1], Alu.max)
                thr = mm
                nc.vector.tensor_tensor(thr, mm, nm, Alu.max)
                # prune: xspr = |x| >= thr ? x : 0, cast to bf16 on write
                thr_b = thr.unsqueeze(2).broadcast_to([P, HALF // 4, 4])
                xspr = spool.tile([P, HALF], bf16, tag="xspr")
                nc.vector._custom_dve(
                    PRUNE24,
                    out=xspr.rearrange("p (g four) -> p g four", four=4),
                    in0=xh.rearrange("p (g four) -> p g four", four=4),
                    in1=thr_b)
                # SBUF->SBUF 128x128 transposes via the DMA xbar:
                # out[p, c, t] = xspr[t, 128c + p].  On the sync queue with
                # the x loads (same producer-side dependency cadence) so
                # they never queue behind PSUM-copy semaphore waits.
                nc.sync.dma_start(out=xt[h], in_=xspr, transpose=True)

            w_pieces = []
            for i in range(NT):
                # one bf16 x load per tile; [d, chunk, tok] transposed
                # pruned activations deposited per half (separate tiles so
                # half-0 matmuls don't wait on the half-1 transpose).
                xh = xin.tile([P, D], bf16, tag="xh")
                nc.sync.dma_start(out=xh, in_=xs_ap[i * P:(i + 1) * P, :])
                xt0 = xtp.tile([P, NCH_H, P], bf16, tag="xt0")
                xt1 = xtp.tile([P, NCH_H, P], bf16, tag="xt1")
                xt = [xt0, xt1]
                for h in range(2):
                    process_span(i, xt, xh, h)

                if i == 0:
                    # weight resident in SBUF as 8 x 1MB piece tiles
                    # [P, 4, OUTF] bf16 (piece j = d-chunks 4j..4j+3) on the
                    # scalar HWDGE queue, created AFTER tile 0's spans.  The
                    # scheduler's cost model serializes all DMAs through one
                    # exclusive resource; small pieces let the ready (and
                    # higher-priority) xbar transposes slot between them
                    # instead of stalling behind one 8MB transfer.  Exactly
                    # 8 pieces: each lands on a distinct DMAHW lane whose
                    # predecessor is an early fast DMA -- with 16 pieces the
                    # later ones lane-wait on the first transposes, which
                    # chain-wait on the pieces, delaying both the weight
                    # completion and the PE start.
                    for j in range(NCH // 4):
                        w_p = wpool.tile([P, 4, OUTF], bf16, tag=f"w{j}")
                        nc.scalar.dma_start(
                            out=w_p, in_=wt_ap[:, 4 * j:4 * j + 4, :])
                        w_pieces.append(w_p)

                # matmul: psum[tok, outf-half] += xt[h][c].T @ wT[h][c].
                # h-outer at 16-matmul burst granularity: both banks' half-0
                # bursts run before any half-1 burst, so the PE can proceed
                # while the half-1 transpose is still in flight.  Each
                # bank's accumulation group stays in contiguous 16-MM bursts
                # (per-matmul bank alternation triggers the PSUM-cycling HAM
                # degradation).
                osb = outp.tile([P, OUTF], bf16, tag="osb")
                pout0 = pso.tile([P, OUTF // 2], f32, tag="ps0", bufs=3)
                pout1 = pso.tile([P, OUTF // 2], f32, tag="ps1", bufs=3)
                pouts = [pout0, pout1]
                for h in range(2):
                    for n in range(2):
                        for c in range(NCH_H):
                            cc = h * NCH_H + c
                            nc.tensor.matmul(
                                pouts[n],
                                xt[h][:, c, :],
                                w_pieces[cc // 4][:, cc % 4,
                                                  n * 512:(n + 1) * 512],
                                start=(h == 0 and c == 0),
                                stop=(h == 1 and c == NCH_H - 1))
                for n in range(2):
                    # PSUM f32 -> SBUF bf16 cast on the ACT copy
                    nc.scalar.copy(osb[:, n * 512:(n + 1) * 512], pouts[n])
                # one bf16 store per tile on the SWDGE queue
                nc.gpsimd.dma_start(out=o_ap[i * P:(i + 1) * P, :], in_=osb)
    nc.compile()
    return nc


def _get_compiled():
    global _compiled
    if _compiled is None:
        _compiled = _build()
    return _compiled


def _fix_ties(x_flat):
    # The device prunes on bf16 values: it keeps elements with
    # |bf16(x)| >= (2nd-largest |bf16(x)| of the group).  Where that
    # decision differs from the reference (fp32 top_k, stable tie-break) --
    # i.e. groups whose 2nd and 3rd magnitudes collapse to the same bf16 --
    # pre-zero the reference-DROPPED elements so the device's threshold
    # test keeps exactly the reference-kept pair.  The zeroed elements are
    # dropped by the reference either way, so values are unaffected.
    import ml_dtypes
    ab = np.abs(x_flat.astype(ml_dtypes.bfloat16).astype(np.float32))
    g = ab.reshape(-1, 4)
    m1 = np.maximum(g[:, 0], g[:, 1]); n1 = np.minimum(g[:, 0], g[:, 1])
    m2 = np.maximum(g[:, 2], g[:, 3]); n2 = np.minimum(g[:, 2], g[:, 3])
    thr = np.maximum(np.minimum(m1, m2), np.maximum(n1, n2))
    third = np.minimum(np.minimum(m1, m2), np.maximum(n1, n2))
    tied = np.flatnonzero(thr == third)
    if len(tied) == 0:
        return x_flat
    x_flat = x_flat.copy()
    gv = x_flat.reshape(-1, 4)
    rows = gv[tied]
    # reference keep-set: top-2 of fp32 |x|, stable order
    order = np.argsort(-np.abs(rows), axis=1, kind="stable")
    np.put_along_axis(rows, order[:, 2:], 0.0, axis=1)
    gv[tied] = rows
    return x_flat


def _prep_inputs(x: np.ndarray, weight: np.ndarray) -> list:
    import ml_dtypes
    x_flat = np.ascontiguousarray(x.reshape(TOK_TOTAL, D), dtype=np.float32)
    x_flat = _fix_ties(x_flat)
    xs16 = np.ascontiguousarray(x_flat.astype(ml_dtypes.bfloat16))
    # weight.T permuted to [p, c, o] with d = 128c + p, so the device DMA
    # reads one contiguous 64KB run per partition.
    wt = np.ascontiguousarray(
        weight.T.astype(ml_dtypes.bfloat16)
        .reshape(NCH, P, OUTF).transpose(1, 0, 2))
    return [{"xs": xs16[c * TOK:(c + 1) * TOK], "wt": wt}
            for c in range(N_CORES)]


def kernel(x: np.ndarray, weight: np.ndarray) -> np.ndarray:
    from concourse.bass_utils import run_bass_kernel_spmd

    nc = _get_compiled()
    in_maps = _prep_inputs(x, weight)
    res = run_bass_kernel_spmd(nc, in_maps, core_ids=list(range(N_CORES)))
    out = np.concatenate([res.results[c]["o"] for c in range(N_CORES)],
                         axis=0).astype(np.float32)
    return out.reshape(BS, SEQ, OUTF)


# revision 59
# speedup vs baseline: 1.1122x; 1.0579x over previous
# Trainium2 Bass kernel: 2:4 structured activation pruning + Linear.
#
#   out = magnitude_prune_2of4(x.reshape(-1, 4096)) @ weight.T
#
# Sharding: data-parallel over the flattened token dim (16384 tokens ->
# 2048/core across 8 cores); weight replicated (host-permuted + bf16 so
# the contraction dim lands on SBUF partitions). No collectives.
#
# Pipeline (PE does ONLY matmuls; transposes run on the DMA xbar):
#   host: x/weight -> bf16 (+ bf16-level tie fix, exact decisions)
#   -> DMA x (bf16) -> DVE |x| pairwise max/min (custom ops) -> DVE
#   compact tree -> per-group-of-4 2nd-max threshold (exact fp32 over the
#   bf16 values) -> DVE prune-select -> DMA-xbar SBUF->SBUF transpose
#   -> PE bf16 matmuls (FWL weight loads) accumulating over 32 d-chunks
#   -> ACT PSUM->SBUF copy w/ bf16 cast -> DMA out (bf16, host-upcast).
#
# Scheduling notes (hardware-trace driven; see the memory file
# trn2-tile-scheduler-dma-serialization for the full story): the tile
# scheduler serializes ALL DMAs through one modeled resource, so the 8MB
# weight ships as 8 x 1MB pieces created after tile 0's spans (exactly 8:
# one per DMAHW lane, so no piece lane-waits on a transpose); PSUM
# accumulation groups stay in contiguous 16-matmul bursts; loads +
# transposes share the sync HWDGE queue, PSUM copies the scalar queue,
# stores the gpsimd SWDGE queue.
import numpy as np

N_CORES = 8
BS, SEQ, D = 4, 4096, 4096
OUTF = 1024
TOK_TOTAL = BS * SEQ
TOK = TOK_TOTAL // N_CORES      # 2048 tokens per core
P = 128                         # SBUF partitions
NT = TOK // P                   # 16 token tiles per core
HALF = D // 2                   # 2048: free-dim half width
NCH = D // P                    # 32 d-chunks of 128
NCH_H = NCH // 2                # 16 d-chunks per half

_compiled = None
_custom_ops = None


def _register_custom_dve():
    # Fused DVE ops (registered into the runtime op table; compiled into the
    # per-NEFF DVE table): pairwise abs-max/abs-min, and the pruning select
    # out = |x| >= thr ? x : 0. Halves DVE work vs stock-op sequences.
    global _custom_ops
    if _custom_ops is not None:
        return _custom_ops
    from concourse import dve_ops as D
    from concourse.dve_spec import Spec, Src0, Src1, Zero, maxx, minn, select, lower
    from concourse.dve_uop import DveOpSpec

    def mk(name, body, reference):
        spec = Spec(body=body, reference=reference)
        shas = {}
        for ver in ("v3", "v4"):
            try:
                u = lower(spec, ver=ver)
                shas[ver] = DveOpSpec(name=name, opcode=1, uops=u,
                                      rd1_en=True).sha(ver)
            except Exception:
                if ver == "v3":
                    raise
        return D.DveOp(name=name, spec=spec, subdim=False, uops_sha=shas)

    absa = maxx(Src0, Zero - Src0)
    absb = maxx(Src1, Zero - Src1)
    ops = (
        mk("ABS_MAX2_ANT", maxx(absa, absb),
           lambda in0, in1, *a: np.maximum(np.abs(in0), np.abs(in1))),
        mk("ABS_MIN2_ANT", minn(absa, absb),
           lambda in0, in1, *a: np.minimum(np.abs(in0), np.abs(in1))),
        mk("PRUNE24_ANT", select(maxx(Src0, Zero - Src0) >= Src1, Src0, Zero),
           lambda in0, in1, *a: np.where(np.abs(in0) >= in1, in0, 0.0)),
    )
    for op in ops:
        if op.name not in D._SUB_OPCODE_FOR_NAME:
            D.OPS.append(op)
            D.CUSTOM_DVE_SPECS[op.name] = op.spec
            D._SUB_OPCODE_FOR_NAME[op.name] = (
                D._CUSTOM_DVE_ROW_BASE + len(D._SUB_OPCODE_FOR_NAME))
    _custom_ops = ops
    return ops


def _build():
    import concourse.tile as tile
    import concourse.mybir as mybir
    from concourse import bacc

    ABS_MAX2, ABS_MIN2, PRUNE24 = _register_custom_dve()
    f32 = mybir.dt.float32
    bf16 = mybir.dt.bfloat16
    Alu = mybir.AluOpType

    nc = bacc.Bacc("TRN2", target_bir_lowering=False, debug=False,
                   num_devices=N_CORES)
    xs_ap = nc.dram_tensor("xs", [TOK, D], bf16, kind="ExternalInput").ap()
    # weight.T host-permuted to [p, c, o] (d = 128c + p) so each partition's
    # DMA source is one contiguous 64KB run -- few big descriptors.
    wt_ap = nc.dram_tensor("wt", [P, NCH, OUTF], bf16,
                           kind="ExternalInput").ap()
    o_ap = nc.dram_tensor("o", [TOK, OUTF], bf16, kind="ExternalOutput").ap()

    with tile.TileContext(nc) as tc:
        with tc.tile_pool(name="wpool", bufs=1) as wpool, \
             tc.tile_pool(name="xin", bufs=3) as xin, \
             tc.tile_pool(name="mwork", bufs=2) as mwork, \
             tc.tile_pool(name="twork", bufs=2) as twork, \
             tc.tile_pool(name="spool", bufs=2) as spool, \
             tc.tile_pool(name="xtp", bufs=2) as xtp, \
             tc.tile_pool(name="outp", bufs=2) as outp, \
             tc.tile_pool(name="pso", bufs=4, space="PSUM") as pso:

            def process_span(i, xsp_full, xh, h):
                # prune x[i-tile, h-half] (bf16; thresholds exact in fp32 on
                # the bf16 values) into xsp_full[:, h-half].
                xh = xh[:, h * HALF:(h + 1) * HALF]
                # pairwise tree: thr = 2nd-largest |x| per group of 4
                x2 = xh.rearrange("p (g two) -> p g two", two=2)
                mx = mwork.tile([P, HALF // 2], f32, tag="mx")
                mn = mwork.tile([P, HALF // 2], f32, tag="mn")
                nc.vector._custom_dve(ABS_MAX2, out=mx,
                                      in0=x2[:, :, 0], in1=x2[:, :, 1])
                nc.vector._custom_dve(ABS_MIN2, out=mn,
                                      in0=x2[:, :, 0], in1=x2[:, :, 1])
                # compact: 2nd-max = max(min of pair-maxes, max of pair-mins)
                mx2 = mx.rearrange("p (g two) -> p g two", two=2)
                mn2 = mn.rearrange("p (g two) -> p g two", two=2)
                mm = twork.tile([P, HALF // 4], f32, tag="mm")
                nm = twork.tile([P, HALF // 4], f32, tag="nm")
                nc.vector.tensor_tensor(mm, mx2[:, :, 0], mx2[:, :, 1], Alu.min)
                nc.vector.tensor_tensor(nm, mn2[:, :, 0], mn2[:, :, 1], Alu.max)
                thr = mm
                nc.vector.tensor_tensor(thr, mm, nm, Alu.max)
                # prune: xspr = |x| >= thr ? x : 0, cast to bf16 on write
                thr_b = thr.unsqueeze(2).broadcast_to([P, HALF // 4, 4])
                xspr = xsp_full[:, h * HALF:(h + 1) * HALF]
                nc.vector._custom_dve(
                    PRUNE24,
                    out=xspr.rearrange("p (g four) -> p g four", four=4),
                    in0=xh.rearrange("p (g four) -> p g four", four=4),
                    in1=thr_b)

            w_pieces = []
            for i in range(NT):
                # one bf16 x load per tile; [d, chunk, tok] transposed
                # pruned activations deposited per half (separate tiles so
                # half-0 matmuls don't wait on the half-1 transpose).
                xh = xin.tile([P, D], bf16, tag="xh")
                nc.sync.dma_start(out=xh, in_=xs_ap[i * P:(i + 1) * P, :])
                xsp_full = spool.tile([P, D], bf16, tag="xspr")
                for h in range(2):
                    process_span(i, xsp_full, xh, h)
                # ONE full-width SBUF->SBUF xbar transpose per tile:
                # out[p, c, t] = xspr[t, 128c + p].  Fewer DMA dispatches in
                # the scheduler's serialized DMA chain than per-half.
                xt = xtp.tile([P, NCH, P], bf16, tag="xt")
                nc.sync.dma_start(out=xt, in_=xsp_full, transpose=True)

                if i == 0:
                    # weight resident in SBUF as 8 x 1MB piece tiles
                    # [P, 4, OUTF] bf16 (piece j = d-chunks 4j..4j+3) on the
                    # scalar HWDGE queue, created AFTER tile 0's spans.  The
                    # scheduler's cost model serializes all DMAs through one
                    # exclusive resource; small pieces let the ready (and
                    # higher-priority) xbar transposes slot between them
                    # instead of stalling behind one 8MB transfer.  Exactly
                    # 8 pieces: each lands on a distinct DMAHW lane whose
                    # predecessor is an early fast DMA -- with 16 pieces the
                    # later ones lane-wait on the first transposes, which
                    # chain-wait on the pieces, delaying both the weight
                    # completion and the PE start.
                    for j in range(NCH // 4):
                        w_p = wpool.tile([P, 4, OUTF], bf16, tag=f"w{j}")
                        nc.scalar.dma_start(
                            out=w_p, in_=wt_ap[:, 4 * j:4 * j + 4, :])
                        w_pieces.append(w_p)

                # matmul: psum[tok, outf-half] += xt[h][c].T @ wT[h][c].
                # h-outer at 16-matmul burst granularity: both banks' half-0
                # bursts run before any half-1 burst, so the PE can proceed
                # while the half-1 transpose is still in flight.  Each
                # bank's accumulation group stays in contiguous 16-MM bursts
                # (per-matmul bank alternation triggers the PSUM-cycling HAM
                # degradation).
                osb = outp.tile([P, OUTF], bf16, tag="osb")
                pout0 = pso.tile([P, OUTF // 2], f32, tag="ps0", bufs=2)
                pout1 = pso.tile([P, OUTF // 2], f32, tag="ps1", bufs=2)
                pouts = [pout0, pout1]
                for n in range(2):
                    for cc in range(NCH):
                        nc.tensor.matmul(
                            pouts[n],
                            xt[:, cc, :],
                            w_pieces[cc // 4][:, cc % 4,
                                              n * 512:(n + 1) * 512],
                            start=(cc == 0), stop=(cc == NCH - 1))
                for n in range(2):
                    # PSUM f32 -> SBUF bf16 cast on the ACT copy
                    nc.scalar.copy(osb[:, n * 512:(n + 1) * 512], pouts[n])
                # one bf16 store per tile on the SWDGE queue
                nc.gpsimd.dma_start(out=o_ap[i * P:(i + 1) * P, :], in_=osb)
    nc.compile()
    return nc


def _get_compiled():
    global _compiled
    if _compiled is None:
        _compiled = _build()
    return _compiled


def _fix_ties(x_flat):
    # The device prunes on bf16 values: it keeps elements with
    # |bf16(x)| >= (2nd-largest |bf16(x)| of the group).  Where that
    # decision differs from the reference (fp32 top_k, stable tie-break) --
    # i.e. groups whose 2nd and 3rd magnitudes collapse to the same bf16 --
    # pre-zero the reference-DROPPED elements so the device's threshold
    # test keeps exactly the reference-kept pair.  The zeroed elements are
    # dropped by the reference either way, so values are unaffected.
    import ml_dtypes
    ab = np.abs(x_flat.astype(ml_dtypes.bfloat16).astype(np.float32))
    g = ab.reshape(-1, 4)
    m1 = np.maximum(g[:, 0], g[:, 1]); n1 = np.minimum(g[:, 0], g[:, 1])
    m2 = np.maximum(g[:, 2], g[:, 3]); n2 = np.minimum(g[:, 2], g[:, 3])
    thr = np.maximum(np.minimum(m1, m2), np.maximum(n1, n2))
    third = np.minimum(np.minimum(m1, m2), np.maximum(n1, n2))
    tied = np.flatnonzero(thr == third)
    if len(tied) == 0:
        return x_flat
    x_flat = x_flat.copy()
    gv = x_flat.reshape(-1, 4)
    rows = gv[tied]
    # reference keep-set: top-2 of fp32 |x|, stable order
    order = np.argsort(-np.abs(rows), axis=1, kind="stable")
    np.put_along_axis(rows, order[:, 2:], 0.0, axis=1)
    gv[tied] = rows
    return x_flat


def _prep_inputs(x: np.ndarray, weight: np.ndarray) -> list:
    import ml_dtypes
    x_flat = np.ascontiguousarray(x.reshape(TOK_TOTAL, D), dtype=np.float32)
    x_flat = _fix_ties(x_flat)
    xs16 = np.ascontiguousarray(x_flat.astype(ml_dtypes.bfloat16))
    # weight.T permuted to [p, c, o] with d = 128c + p, so the device DMA
    # reads one contiguous 64KB run per partition.
    wt = np.ascontiguousarray(
        weight.T.astype(ml_dtypes.bfloat16)
        .reshape(NCH, P, OUTF).transpose(1, 0, 2))
    return [{"xs": xs16[c * TOK:(c + 1) * TOK], "wt": wt}
            for c in range(N_CORES)]


def kernel(x: np.ndarray, weight: np.ndarray) -> np.ndarray:
    from concourse.bass_utils import run_bass_kernel_spmd

    nc = _get_compiled()
    in_maps = _prep_inputs(x, weight)
    res = run_bass_kernel_spmd(nc, in_maps, core_ids=list(range(N_CORES)))
    out = np.concatenate([res.results[c]["o"] for c in range(N_CORES)],
                         axis=0).astype(np.float32)
    return out.reshape(BS, SEQ, OUTF)


# revision 60
# speedup vs baseline: 1.1446x; 1.0291x over previous
# Trainium2 Bass kernel: 2:4 structured activation pruning + Linear.
#
#   out = magnitude_prune_2of4(x.reshape(-1, 4096)) @ weight.T
#
# Sharding: data-parallel over the flattened token dim (16384 tokens ->
# 2048/core across 8 cores); weight replicated (host-permuted + bf16 so
# the contraction dim lands on SBUF partitions). No collectives.
#
# Pipeline (PE does ONLY matmuls; transposes run on the DMA xbar):
#   host: x/weight -> bf16 (+ bf16-level tie fix, exact decisions)
#   -> DMA x (bf16) -> DVE |x| pairwise max/min (custom ops) -> DVE
#   compact tree -> per-group-of-4 2nd-max threshold (exact fp32 over the
#   bf16 values) -> DVE prune-select -> DMA-xbar SBUF->SBUF transpose
#   -> PE bf16 matmuls (FWL weight loads) accumulating over 32 d-chunks
#   -> ACT PSUM->SBUF copy w/ bf16 cast -> DMA out (bf16, host-upcast).
#
# Scheduling notes (hardware-trace driven; see the memory file
# trn2-tile-scheduler-dma-serialization for the full story): the tile
# scheduler serializes ALL DMAs through one modeled resource, so the 8MB
# weight ships as 8 x 1MB pieces created after tile 0's spans (exactly 8:
# one per DMAHW lane, so no piece lane-waits on a transpose); PSUM
# accumulation groups stay in contiguous 16-matmul bursts; loads +
# transposes share the sync HWDGE queue, PSUM copies the scalar queue,
# stores the gpsimd SWDGE queue.
import numpy as np

N_CORES = 8
BS, SEQ, D = 4, 4096, 4096
OUTF = 1024
TOK_TOTAL = BS * SEQ
TOK = TOK_TOTAL // N_CORES      # 2048 tokens per core
P = 128                         # SBUF partitions
NT = TOK // P                   # 16 token tiles per core
HALF = D // 2                   # 2048: free-dim half width
NCH = D // P                    # 32 d-chunks of 128
NCH_H = NCH // 2                # 16 d-chunks per half

_compiled = None
_custom_ops = None


def _register_custom_dve():
    # Fused DVE ops (registered into the runtime op table; compiled into the
    # per-NEFF DVE table): pairwise abs-max/abs-min, and the pruning select
    # out = |x| >= thr ? x : 0. Halves DVE work vs stock-op sequences.
    global _custom_ops
    if _custom_ops is not None:
        return _custom_ops
    from concourse import dve_ops as D
    from concourse.dve_spec import Spec, Src0, Src1, Zero, maxx, minn, select, lower
    from concourse.dve_uop import DveOpSpec

    def mk(name, body, reference):
        spec = Spec(body=body, reference=reference)
        shas = {}
        for ver in ("v3", "v4"):
            try:
                u = lower(spec, ver=ver)
                shas[ver] = DveOpSpec(name=name, opcode=1, uops=u,
                                      rd1_en=True).sha(ver)
            except Exception:
                if ver == "v3":
                    raise
        return D.DveOp(name=name, spec=spec, subdim=False, uops_sha=shas)

    absa = maxx(Src0, Zero - Src0)
    absb = maxx(Src1, Zero - Src1)
    ops = (
        mk("ABS_MAX2_ANT", maxx(absa, absb),
           lambda in0, in1, *a: np.maximum(np.abs(in0), np.abs(in1))),
        mk("ABS_MIN2_ANT", minn(absa, absb),
           lambda in0, in1, *a: np.minimum(np.abs(in0), np.abs(in1))),
        mk("PRUNE24_ANT", select(maxx(Src0, Zero - Src0) >= Src1, Src0, Zero),
           lambda in0, in1, *a: np.where(np.abs(in0) >= in1, in0, 0.0)),
    )
    for op in ops:
        if op.name not in D._SUB_OPCODE_FOR_NAME:
            D.OPS.append(op)
            D.CUSTOM_DVE_SPECS[op.name] = op.spec
            D._SUB_OPCODE_FOR_NAME[op.name] = (
                D._CUSTOM_DVE_ROW_BASE + len(D._SUB_OPCODE_FOR_NAME))
    _custom_ops = ops
    return ops


def _build():
    import concourse.tile as tile
    import concourse.mybir as mybir
    from concourse import bacc

    ABS_MAX2, ABS_MIN2, PRUNE24 = _register_custom_dve()
    f32 = mybir.dt.float32
    bf16 = mybir.dt.bfloat16
    Alu = mybir.AluOpType

    nc = bacc.Bacc("TRN2", target_bir_lowering=False, debug=False,
                   num_devices=N_CORES)
    xs_ap = nc.dram_tensor("xs", [TOK, D], bf16, kind="ExternalInput").ap()
    # weight.T host-permuted to [p, c, o] (d = 128c + p) so each partition's
    # DMA source is one contiguous 64KB run -- few big descriptors.
    wt_ap = nc.dram_tensor("wt", [P, NCH, OUTF], bf16,
                           kind="ExternalInput").ap()
    o_ap = nc.dram_tensor("o", [TOK, OUTF], bf16, kind="ExternalOutput").ap()

    with tile.TileContext(nc) as tc:
        with tc.tile_pool(name="wpool", bufs=1) as wpool, \
             tc.tile_pool(name="xin", bufs=3) as xin, \
             tc.tile_pool(name="mwork", bufs=2) as mwork, \
             tc.tile_pool(name="twork", bufs=2) as twork, \
             tc.tile_pool(name="spool", bufs=2) as spool, \
             tc.tile_pool(name="xtp", bufs=2) as xtp, \
             tc.tile_pool(name="outp", bufs=2) as outp, \
             tc.tile_pool(name="pso", bufs=4, space="PSUM") as pso:

            def process_span(i, xsp_full, xh, h):
                # prune x[i-tile, h-half] (bf16; thresholds exact in fp32 on
                # the bf16 values) into xsp_full[:, h-half].
                xh = xh[:, h * HALF:(h + 1) * HALF]
                # pairwise tree: thr = 2nd-largest |x| per group of 4
                x2 = xh.rearrange("p (g two) -> p g two", two=2)
                mx = mwork.tile([P, HALF // 2], f32, tag="mx")
                mn = mwork.tile([P, HALF // 2], f32, tag="mn")
                nc.vector._custom_dve(ABS_MAX2, out=mx,
                                      in0=x2[:, :, 0], in1=x2[:, :, 1])
                nc.vector._custom_dve(ABS_MIN2, out=mn,
                                      in0=x2[:, :, 0], in1=x2[:, :, 1])
                # compact: 2nd-max = max(min of pair-maxes, max of pair-mins)
                mx2 = mx.rearrange("p (g two) -> p g two", two=2)
                mn2 = mn.rearrange("p (g two) -> p g two", two=2)
                mm = twork.tile([P, HALF // 4], f32, tag="mm")
                nm = twork.tile([P, HALF // 4], f32, tag="nm")
                nc.vector.tensor_tensor(mm, mx2[:, :, 0], mx2[:, :, 1], Alu.min)
                nc.vector.tensor_tensor(nm, mn2[:, :, 0], mn2[:, :, 1], Alu.max)
                thr = mm
                nc.vector.tensor_tensor(thr, mm, nm, Alu.max)
                # prune: xspr = |x| >= thr ? x : 0, cast to bf16 on write
                thr_b = thr.unsqueeze(2).broadcast_to([P, HALF // 4, 4])
                xspr = xsp_full[:, h * HALF:(h + 1) * HALF]
                nc.vector._custom_dve(
                    PRUNE24,
                    out=xspr.rearrange("p (g four) -> p g four", four=4),
                    in0=xh.rearrange("p (g four) -> p g four", four=4),
                    in1=thr_b)

            w_pieces = []
            for i in range(NT):
                # one bf16 x load per tile; [d, chunk, tok] transposed
                # pruned activations deposited per half (separate tiles so
                # half-0 matmuls don't wait on the half-1 transpose).
                xh = xin.tile([P, D], bf16, tag="xh")
                nc.sync.dma_start(out=xh, in_=xs_ap[i * P:(i + 1) * P, :])
                xsp_full = spool.tile([P, D], bf16, tag="xspr")
                for h in range(2):
                    process_span(i, xsp_full, xh, h)
                if i == 0:
                    # tile 0 only: per-half transposes into separate tiles
                    # so the first matmul burst starts ~15us earlier, while
                    # the 8MB weight is still streaming anyway.
                    xt0 = xtp.tile([P, NCH_H, P], bf16, tag="xt0")
                    xt1 = xtp.tile([P, NCH_H, P], bf16, tag="xt1")
                    nc.sync.dma_start(out=xt0, in_=xsp_full[:, :HALF],
                                      transpose=True)
                    nc.sync.dma_start(out=xt1, in_=xsp_full[:, HALF:],
                                      transpose=True)
                    spans = [(xt0, NCH_H, True, False),
                             (xt1, NCH_H, False, True)]
                else:
                    # ONE full-width SBUF->SBUF xbar transpose per tile:
                    # out[p, c, t] = xspr[t, 128c + p].  One clean per-tile
                    # dependency and fewer DMA dispatches in the scheduler's
                    # serialized chain -- this is what makes the steady
                    # state gap-free.
                    xt = xtp.tile([P, NCH, P], bf16, tag="xt")
                    nc.sync.dma_start(out=xt, in_=xsp_full, transpose=True)
                    spans = [(xt, NCH, True, True)]

                if i == 0:
                    # weight resident in SBUF as 8 x 1MB piece tiles
                    # [P, 4, OUTF] bf16 (piece j = d-chunks 4j..4j+3) on the
                    # scalar HWDGE queue, created AFTER tile 0's spans.  The
                    # scheduler's cost model serializes all DMAs through one
                    # exclusive resource; small pieces let the ready (and
                    # higher-priority) xbar transposes slot between them
                    # instead of stalling behind one 8MB transfer.  Exactly
                    # 8 pieces: each lands on a distinct DMAHW lane whose
                    # predecessor is an early fast DMA -- with 16 pieces the
                    # later ones lane-wait on the first transposes, which
                    # chain-wait on the pieces, delaying both the weight
                    # completion and the PE start.
                    for j in range(NCH // 4):
                        w_p = wpool.tile([P, 4, OUTF], bf16, tag=f"w{j}")
                        nc.scalar.dma_start(
                            out=w_p, in_=wt_ap[:, 4 * j:4 * j + 4, :])
                        w_pieces.append(w_p)

                # matmul: psum[tok, outf-half] += xt[h][c].T @ wT[h][c].
                # h-outer at 16-matmul burst granularity: both banks' half-0
                # bursts run before any half-1 burst, so the PE can proceed
                # while the half-1 transpose is still in flight.  Each
                # bank's accumulation group stays in contiguous 16-MM bursts
                # (per-matmul bank alternation triggers the PSUM-cycling HAM
                # degradation).
                osb = outp.tile([P, OUTF], bf16, tag="osb")
                pout0 = pso.tile([P, OUTF // 2], f32, tag="ps0", bufs=2)
                pout1 = pso.tile([P, OUTF // 2], f32, tag="ps1", bufs=2)
                pouts = [pout0, pout1]
                cc0 = 0
                for xt_s, nch_s, first, last in spans:
                    for n in range(2):
                        for c in range(nch_s):
                            cc = cc0 + c
                            nc.tensor.matmul(
                                pouts[n],
                                xt_s[:, c, :],
                                w_pieces[cc // 4][:, cc % 4,
                                                  n * 512:(n + 1) * 512],
                                start=(first and c == 0),
                                stop=(last and c == nch_s - 1))
                    cc0 += nch_s
                for n in range(2):
                    # PSUM f32 -> SBUF bf16 cast on the ACT copy
                    nc.scalar.copy(osb[:, n * 512:(n + 1) * 512], pouts[n])
                # one bf16 store per tile on the SWDGE queue
                nc.gpsimd.dma_start(out=o_ap[i * P:(i + 1) * P, :], in_=osb)
    nc.compile()
    return nc


def _get_compiled():
    global _compiled
    if _compiled is None:
        _compiled = _build()
    return _compiled


def _fix_ties(x_flat):
    # The device prunes on bf16 values: it keeps elements with
    # |bf16(x)| >= (2nd-largest |bf16(x)| of the group).  Where that
    # decision differs from the reference (fp32 top_k, stable tie-break) --
    # i.e. groups whose 2nd and 3rd magnitudes collapse to the same bf16 --
    # pre-zero the reference-DROPPED elements so the device's threshold
    # test keeps exactly the reference-kept pair.  The zeroed elements are
    # dropped by the reference either way, so values are unaffected.
    import ml_dtypes
    ab = np.abs(x_flat.astype(ml_dtypes.bfloat16).astype(np.float32))
    g = ab.reshape(-1, 4)
    m1 = np.maximum(g[:, 0], g[:, 1]); n1 = np.minimum(g[:, 0], g[:, 1])
    m2 = np.maximum(g[:, 2], g[:, 3]); n2 = np.minimum(g[:, 2], g[:, 3])
    thr = np.maximum(np.minimum(m1, m2), np.maximum(n1, n2))
    third = np.minimum(np.minimum(m1, m2), np.maximum(n1, n2))
    tied = np.flatnonzero(thr == third)
    if len(tied) == 0:
        return x_flat
    x_flat = x_flat.copy()
    gv = x_flat.reshape(-1, 4)
    rows = gv[tied]
    # reference keep-set: top-2 of fp32 |x|, stable order
    order = np.argsort(-np.abs(rows), axis=1, kind="stable")
    np.put_along_axis(rows, order[:, 2:], 0.0, axis=1)
    gv[tied] = rows
    return x_flat


def _prep_inputs(x: np.ndarray, weight: np.ndarray) -> list:
    import ml_dtypes
    x_flat = np.ascontiguousarray(x.reshape(TOK_TOTAL, D), dtype=np.float32)
    x_flat = _fix_ties(x_flat)
    xs16 = np.ascontiguousarray(x_flat.astype(ml_dtypes.bfloat16))
    # weight.T permuted to [p, c, o] with d = 128c + p, so the device DMA
    # reads one contiguous 64KB run per partition.
    wt = np.ascontiguousarray(
        weight.T.astype(ml_dtypes.bfloat16)
        .reshape(NCH, P, OUTF).transpose(1, 0, 2))
    return [{"xs": xs16[c * TOK:(c + 1) * TOK], "wt": wt}
            for c in range(N_CORES)]


def kernel(x: np.ndarray, weight: np.ndarray) -> np.ndarray:
    from concourse.bass_utils import run_bass_kernel_spmd

    nc = _get_compiled()
    in_maps = _prep_inputs(x, weight)
    res = run_bass_kernel_spmd(nc, in_maps, core_ids=list(range(N_CORES)))
    out = np.concatenate([res.results[c]["o"] for c in range(N_CORES)],
                         axis=0).astype(np.float32)
    return out.reshape(BS, SEQ, OUTF)
